# revision 32
# baseline (speedup 1.0000x reference)
"""RNN-T Joiner kernel for Trainium2 (8 NeuronCores, SPMD data-parallel over B).

Computation (per batch element b, handled by core b):
    enc  = encoder_output[b] @ W_enc.T + b_enc        # (T, J)
    pred = predictor_output[b] @ W_pred.T + b_pred    # (U, J)
    h    = relu(enc[:, None, :] + pred[None, :, :])   # (T, U, J)
    out  = h @ W_out.T + b_out                        # (T, U, V)

Device layout: joint dim j lives on SBUF partitions. Per u, the scalar (ACT)
engine builds h[j, t] = relu(enc[j, t] + pred[j, u]) in one fused activation
per j-chunk, writing bf16. The bf16 h slices [j=128, t=128] are the stationary
operand against bf16 W_out.T chunks (N = 342/342/341, one PSUM bank each) —
bf16 runs the PE at 1 cycle/row vs 4 for fp32, and the 2e-2 harness tolerance
leaves ~5x headroom over the measured ~3.5e-3 error. Bias b_out is added
during the PSUM->SBUF copy on DVE. All operand transposes are host-side numpy
marshalling.
"""

import os
import sys

import numpy as np

for _p in (
    "/opt/trn_rl_repo",
    os.path.join(os.path.expanduser("~"), ".axon_site", "_ro", "trn_rl_repo"),
):
    if os.path.isdir(_p) and _p not in sys.path:
        sys.path.append(_p)

from contextlib import ExitStack

import ml_dtypes

import concourse.bass as bass
import concourse.tile as tile
from concourse import mybir
from concourse.bass_utils import run_bass_kernel_spmd

FP = mybir.dt.float32
BF = mybir.dt.bfloat16
B, T, U = 8, 256, 64
ENC_DIM, PRED_DIM, JOINT_DIM, OUT_DIM = 512, 640, 640, 1025
N_CORES = 8
P = 128
KE = ENC_DIM // P   # 4  contraction tiles for enc projection
KP = PRED_DIM // P  # 5  contraction tiles for pred projection
KJ = JOINT_DIM // P # 5  contraction tiles for the final matmul
TH = T // P         # 2  t-halves per u
CHUNKS = [(0, 342), (342, 342), (684, 341)]  # N-chunks of OUT_DIM, each <= 1 PSUM bank


def _emit(ctx, tc, ce_t, cp_t, wo_t, bias_j, b_out, out):
    # The cayman LDWEIGHTS ISA struct only has room for ONE sync wait, so every
    # PE matmul must depend on at most one semaphore. Each projection matmul's
    # two operands (weight k-slice + input k-slice) arrive via a single DMA of
    # a host-concatenated [P, 640+T] tile. All loads ride the single SP HWDGE
    # queue (bringing up a second HWDGE queue delays the first transfer on
    # both queues by ~6.5us), ordered by first use. h construction lives on
    # the scalar (ACT) engine and PSUM->SBUF copies on DVE;
    # _split_multi_waits legalizes any instruction with >1 waits.
    nc = tc.nc
    consts = ctx.enter_context(tc.tile_pool(name="consts", bufs=1))
    wo = [consts.tile([P, OUT_DIM], BF, name=f"wo{k}", tag=f"wo{k}") for k in range(KJ)]
    ce = [consts.tile([P, JOINT_DIM + T], BF, name=f"ce{k}", tag=f"ce{k}") for k in range(KE)]
    cp = [consts.tile([P, JOINT_DIM + U], BF, name=f"cp{k}", tag=f"cp{k}") for k in range(KP)]
    bj = [consts.tile([P, 1], FP, name=f"bj{k}", tag=f"bj{k}") for k in range(KJ)]
    bo = consts.tile([P, OUT_DIM], FP, name="bo", tag="bo")
    enc_sb = [consts.tile([P, T], FP, name=f"enc{j}", tag=f"enc{j}") for j in range(KJ)]
    pred_sb = [consts.tile([P, U], FP, name=f"pred{j}", tag=f"pred{j}") for j in range(KJ)]

    for k in range(KE):
        nc.sync.dma_start(out=ce[k][:], in_=ce_t[k * P:(k + 1) * P, :])
    for k in range(KP):
        nc.sync.dma_start(out=cp[k][:], in_=cp_t[k * P:(k + 1) * P, :])
    for k in range(KJ):
        nc.sync.dma_start(out=bj[k][:], in_=bias_j[k * P:(k + 1) * P, :])
    for k in range(KJ):
        nc.sync.dma_start(out=wo[k][:], in_=wo_t[k * P:(k + 1) * P, :])
    nc.sync.dma_start(out=bo[:], in_=b_out[:, :])

    # One PSUM pool for the whole kernel: pse/psp (bufs=1) + ps0..2 (bufs=2)
    # = exactly 8 banks, all disjoint, so no PSUM bank-reuse wait ever lands
    # on a matmul (which could only carry a single sync wait).
    mp = ctx.enter_context(tc.tile_pool(name="mp", bufs=2, space="PSUM"))

    # Projections: enc_j[j, t] (bias deferred) and pred_j[j, u] (+ b_enc + b_pred).
    for j in range(KJ):
        pse = mp.tile([P, T], FP, name="pse", tag="pse", bufs=1)
        for k in range(KE):
            nc.tensor.matmul(pse[:], ce[k][:, j * P:(j + 1) * P],
                             ce[k][:, JOINT_DIM:], start=(k == 0), stop=(k == KE - 1))
        nc.vector.tensor_copy(enc_sb[j][:], pse[:])
        psp = mp.tile([P, U], FP, name="psp", tag="psp", bufs=1)
        for k in range(KP):
            nc.tensor.matmul(psp[:], cp[k][:, j * P:(j + 1) * P],
                             cp[k][:, JOINT_DIM:], start=(k == 0), stop=(k == KP - 1))
        nc.vector.tensor_scalar(pred_sb[j][:], psp[:], bj[j][:], None,
                                mybir.AluOpType.add)

    hp = ctx.enter_context(tc.tile_pool(name="hp", bufs=3))
    op = ctx.enter_context(tc.tile_pool(name="op", bufs=3))
    for u in range(U):
        hs = []
        for k in range(KJ):
            h = hp.tile([P, T], BF, name=f"h{k}", tag=f"h{k}")
            # h = relu(enc[j, :] + pred[j, u]) in one fused ACT op, bf16 out
            nc.scalar.activation(h[:], enc_sb[k][:],
                                 mybir.ActivationFunctionType.Relu,
                                 bias=pred_sb[k][:, u:u + 1], scale=1.0)
            hs.append(h)
        for th in range(TH):
            pss = [mp.tile([P, n], FP, name=f"ps{c}", tag=f"ps{c}") for c, (o, n) in enumerate(CHUNKS)]
            for k in range(KJ):
                hk = hs[k][:, th * P:(th + 1) * P]
                for c, (o, n) in enumerate(CHUNKS):
                    nc.tensor.matmul(pss[c][:], hk, wo[k][:, o:o + n],
                                     start=(k == 0), stop=(k == KJ - 1))
            osb = op.tile([P, OUT_DIM], FP, name="osb", tag="osb")
            for c, (o, n) in enumerate(CHUNKS):
                nc.vector.tensor_add(osb[:, o:o + n], pss[c][:], bo[:, o:o + n])
            nc.sync.dma_start(out=out[th * P:(th + 1) * P, u], in_=osb[:])


def _split_multi_waits(nc):
    """Legalize for walrus builds whose ISA structs carry at most ONE sync wait
    per instruction: move extra waits onto same-engine NoOps inserted right
    before the instruction (engine program order makes that equivalent)."""
    import bass_rust
    n_split = 0
    for fn in nc.m.functions:
        for bb in fn.blocks:
            insts = bb.instructions
            out = []
            for inst in insts:
                si = inst.sync_info
                waits = list(si.on_wait) if si is not None else []
                if len(waits) > 1:
                    for wi, w in enumerate(waits[:-1]):
                        out.append(mybir.InstNoOp(
                            name=f"{inst.name}-w{wi}", engine=inst.engine,
                            sync_info=bass_rust.SyncInfo(on_wait=[w], on_update=[])))
                    inst.sync_info = bass_rust.SyncInfo(
                        on_wait=[waits[-1]], on_update=list(si.on_update))
                    n_split += 1
                out.append(inst)
            if len(out) != len(insts):
                bb.instructions = out
    return n_split


_NC = None


def _build_nc(reps=1):
    nc = bass.Bass()
    ce_t = nc.declare_dram_parameter("ce_t", [ENC_DIM, JOINT_DIM + T], BF, isOutput=False)
    cp_t = nc.declare_dram_parameter("cp_t", [PRED_DIM, JOINT_DIM + U], BF, isOutput=False)
    wo_t = nc.declare_dram_parameter("wo_t", [JOINT_DIM, OUT_DIM], BF, isOutput=False)
    bias_j = nc.declare_dram_parameter("bias_j", [JOINT_DIM, 1], FP, isOutput=False)
    b_out = nc.declare_dram_parameter("b_out", [P, OUT_DIM], FP, isOutput=False)
    out = nc.declare_dram_parameter("out", [T, U, OUT_DIM], FP, isOutput=True)
    with tile.TileContext(nc) as tc:
        with ExitStack() as ctx:
            if reps == 1:
                _emit(ctx, tc, ce_t[:], cp_t[:], wo_t[:], bias_j[:], b_out[:], out[:])
            else:
                with tc.For_i(0, reps, 1):
                    _emit(ctx, tc, ce_t[:], cp_t[:], wo_t[:], bias_j[:], b_out[:], out[:])
    _split_multi_waits(nc)
    return nc


def _get_nc():
    global _NC
    if _NC is None:
        _NC = _build_nc()
    return _NC


def make_in_maps(encoder_output, predictor_output, W_enc, b_enc, W_pred, b_pred,
                 W_out, b_out):
    f32 = np.float32
    bf16 = ml_dtypes.bfloat16
    enc = np.asarray(encoder_output, f32)
    pred = np.asarray(predictor_output, f32)
    we_t = np.asarray(W_enc, f32).T     # [ENC_DIM, JOINT_DIM]
    wp_t = np.asarray(W_pred, f32).T    # [PRED_DIM, JOINT_DIM]
    wo_t = np.ascontiguousarray(np.asarray(W_out, f32).T.astype(bf16))
    bias_j = np.ascontiguousarray(
        (np.asarray(b_enc, f32) + np.asarray(b_pred, f32)).reshape(JOINT_DIM, 1))
    bo_b = np.ascontiguousarray(
        np.broadcast_to(np.asarray(b_out, f32)[None, :], (P, OUT_DIM)))
    in_maps = []
    for b in range(B):
        ce_t = np.ascontiguousarray(np.hstack([we_t, enc[b].T]).astype(bf16))   # [512, 640+256]
        cp_t = np.ascontiguousarray(np.hstack([wp_t, pred[b].T]).astype(bf16))  # [640, 640+64]
        in_maps.append({
            "ce_t": ce_t,
            "cp_t": cp_t,
            "wo_t": wo_t,
            "bias_j": bias_j,
            "b_out": bo_b,
        })
    return in_maps


def run(in_maps, **kwargs):
    return run_bass_kernel_spmd(_get_nc(), in_maps, list(range(N_CORES)), **kwargs)


def kernel(**inputs):
    res = run(make_in_maps(**inputs))
    return np.stack([res.results[i]["out"] for i in range(N_CORES)], axis=0)
